# revision 3
# baseline (speedup 1.0000x reference)
"""Data-parallel EGNN message-passing kernel for 8 Trainium2 NeuronCores.

Strategy (per sharding hint): pure data parallelism. The batch B=8192 is
split across the 8 cores (1024 molecules each); the small MLP/EGNN weights
are replicated. All graph ops (kNN top-k, gather, edge MLP, aggregation,
node MLP, pooling, head) are per-molecule, so shards are independent and no
collectives are needed. Executed on the NeuronCores via jax pmap.
"""

import numpy as np
import jax
import jax.numpy as jnp
from functools import partial

B, N, C, K = 8192, 29, 6, 6
D = 2 * C            # 12
M_DIM = 32
EIN = 2 * D + 1      # 25
H = 32
IDX0, IDX1 = 0, 2
NMAX = 29
NCORES = 8


def _forward(x, context, mask, e_w1, e_b1, e_w2, e_b2, g_w, g_b,
             n_w1, n_b1, n_w2, n_b2, h_w1, h_b1, h_w2, h_b2):
    # x: [b, N, C], context: [b, N, 3], mask: [1, N] bool
    feats = jnp.concatenate([x, x], axis=-1)                      # [b,N,D]
    mask_b = jnp.broadcast_to(mask, (x.shape[0], mask.shape[1]))  # [b,N]

    coors = context
    rel = coors[:, :, None, :] - coors[:, None, :, :]             # [b,N,N,3]
    dist = jnp.sum(rel * rel, axis=-1)                            # [b,N,N]
    pair_mask = mask_b[:, :, None] & mask_b[:, None, :]
    ranking = jnp.where(pair_mask, dist, 1e5)
    # Sort-free exact top-K: iteratively extract the max of -ranking with
    # first-occurrence tie-break (matches lax.top_k semantics).
    cur = -ranking
    oh_slots = []
    for _ in range(K):
        mx = jnp.max(cur, axis=-1, keepdims=True)
        eq = (cur == mx)
        first = eq & (jnp.cumsum(eq.astype(jnp.int32), axis=-1) == 1)
        oh_slots.append(first.astype(x.dtype))
        cur = jnp.where(first, -jnp.inf, cur)
    onehot = jnp.stack(oh_slots, axis=2)                          # [b,N,K,N]
    rel_dist = jnp.einsum("bikn,bin->bik", onehot, dist)[..., None]
    feats_j = jnp.einsum("bikn,bnd->bikd", onehot, feats)
    feats_i = jnp.broadcast_to(feats[:, :, None, :], feats_j.shape)
    edge_in = jnp.concatenate([feats_i, feats_j, rel_dist], axis=-1)
    m_ij = jax.nn.silu(jax.nn.silu(edge_in @ e_w1 + e_b1) @ e_w2 + e_b2)
    m_ij = m_ij * jax.nn.sigmoid(m_ij @ g_w + g_b)
    mask_j = jnp.einsum("bikn,bn->bik", onehot, mask_b.astype(feats.dtype)) > 0.5
    edge_mask = mask_b[:, :, None] & mask_j
    m_ij = jnp.where(edge_mask[..., None], m_ij, 0.0)
    m_i = jnp.sum(m_ij, axis=-2)                                  # [b,N,M]
    node_in = jnp.concatenate([feats, m_i], axis=-1)
    feats = jax.nn.silu(node_in @ n_w1 + n_b1) @ n_w2 + n_b2 + feats

    mf = mask_b.astype(feats.dtype)
    pooled = jnp.sum(feats * mf[..., None], axis=1) / jnp.sum(mf, axis=1, keepdims=True)
    out = jax.nn.relu(pooled @ h_w1 + h_b1) @ h_w2 + h_b2          # [b, 2*D]
    out = out.reshape(x.shape[0], IDX1 - IDX0, D)
    return jnp.pad(out, ((0, 0), (IDX0, NMAX - IDX1), (0, 0)))     # [b,29,12]


_WNAMES = ("e_w1", "e_b1", "e_w2", "e_b2", "g_w", "g_b",
           "n_w1", "n_b1", "n_w2", "n_b2", "h_w1", "h_b1", "h_w2", "h_b2")

_pmapped = None


def _get_pmapped():
    global _pmapped
    if _pmapped is None:
        # x/context sharded on axis 0 (device axis); mask + weights replicated.
        _pmapped = jax.pmap(
            _forward,
            in_axes=(0, 0, None) + (None,) * len(_WNAMES),
            devices=jax.devices()[:NCORES],
        )
    return _pmapped


def kernel(**inputs) -> np.ndarray:
    x = np.asarray(inputs["x"], dtype=np.float32)
    context = np.asarray(inputs["context"], dtype=np.float32)
    mask = np.asarray(inputs["mask"])
    ws = [np.asarray(inputs[n], dtype=np.float32) for n in _WNAMES]

    bl = B // NCORES
    xs = x.reshape(NCORES, bl, N, C)
    cs = context.reshape(NCORES, bl, N, 3)

    fn = _get_pmapped()
    out = fn(xs, cs, mask, *ws)                    # [8, bl, 29, 12]
    out = np.asarray(out, dtype=np.float32).reshape(B, NMAX, D)
    return out


# revision 4
# speedup vs baseline: 5.3077x; 5.3077x over previous
"""Data-parallel EGNN message-passing kernel for 8 Trainium2 NeuronCores.

Strategy (per sharding hint): pure data parallelism. The batch B=8192 is
split across the 8 cores (1024 molecules each); the small MLP/EGNN weights
are replicated. All graph ops (kNN top-k, gather, edge MLP, aggregation,
node MLP, pooling, head) are per-molecule, so shards are independent and no
collectives are needed. Executed on the NeuronCores via jax pmap.
"""

import numpy as np
import jax
import jax.numpy as jnp
from functools import partial

B, N, C, K = 8192, 29, 6, 6
D = 2 * C            # 12
M_DIM = 32
EIN = 2 * D + 1      # 25
H = 32
IDX0, IDX1 = 0, 2
NMAX = 29
NCORES = 8


def _forward(x, context, mask, e_w1, e_b1, e_w2, e_b2, g_w, g_b,
             n_w1, n_b1, n_w2, n_b2, h_w1, h_b1, h_w2, h_b2):
    # x: [b, N, C], context: [b, N, 3], mask: [1, N] bool
    feats = jnp.concatenate([x, x], axis=-1)                      # [b,N,D]
    mask_b = jnp.broadcast_to(mask, (x.shape[0], mask.shape[1]))  # [b,N]

    coors = context
    rel = coors[:, :, None, :] - coors[:, None, :, :]             # [b,N,N,3]
    dist = jnp.sum(rel * rel, axis=-1)                            # [b,N,N]
    pair_mask = mask_b[:, :, None] & mask_b[:, None, :]
    ranking = jnp.where(pair_mask, dist, 1e5)
    # Sort-free exact top-K: iteratively extract the max of -ranking with
    # first-occurrence tie-break (matches lax.top_k semantics).
    cur = -ranking
    oh_slots = []
    for _ in range(K):
        mx = jnp.max(cur, axis=-1, keepdims=True)
        eq = (cur == mx)
        first = eq & (jnp.cumsum(eq.astype(jnp.int32), axis=-1) == 1)
        oh_slots.append(first.astype(x.dtype))
        cur = jnp.where(first, -jnp.inf, cur)
    onehot = jnp.stack(oh_slots, axis=2)                          # [b,N,K,N]
    rel_dist = jnp.einsum("bikn,bin->bik", onehot, dist)[..., None]
    # Fold edge_in concat into split matmuls: edge_in @ e_w1 =
    #   feats_i @ A + feats_j @ B + rel_dist * c   (A,B,c = rows of e_w1)
    A, Bw, cw = e_w1[:D], e_w1[D:2 * D], e_w1[2 * D:]
    U = feats @ A + e_b1                                          # [b,N,2EIN]
    V = jnp.einsum("bikn,bnh->bikh", onehot, feats @ Bw)          # [b,N,K,2EIN]
    h1 = jax.nn.silu(U[:, :, None, :] + V + rel_dist * cw[0])
    m_ij = jax.nn.silu(h1 @ e_w2 + e_b2)
    m_ij = m_ij * jax.nn.sigmoid(m_ij @ g_w + g_b)
    mask_j = jnp.einsum("bikn,bn->bik", onehot, mask_b.astype(feats.dtype)) > 0.5
    edge_mask = mask_b[:, :, None] & mask_j
    m_ij = jnp.where(edge_mask[..., None], m_ij, 0.0)
    m_i = jnp.sum(m_ij, axis=-2)                                  # [b,N,M]
    # node_in concat folded likewise.
    node_pre = feats @ n_w1[:D] + m_i @ n_w1[D:] + n_b1
    feats = jax.nn.silu(node_pre) @ n_w2 + n_b2 + feats

    mf = mask_b.astype(feats.dtype)
    pooled = jnp.sum(feats * mf[..., None], axis=1) / jnp.sum(mf, axis=1, keepdims=True)
    out = jax.nn.relu(pooled @ h_w1 + h_b1) @ h_w2 + h_b2          # [b, 2*D]
    out = out.reshape(x.shape[0], IDX1 - IDX0, D)
    return jnp.pad(out, ((0, 0), (IDX0, NMAX - IDX1), (0, 0)))     # [b,29,12]


_WNAMES = ("e_w1", "e_b1", "e_w2", "e_b2", "g_w", "g_b",
           "n_w1", "n_b1", "n_w2", "n_b2", "h_w1", "h_b1", "h_w2", "h_b2")

_pmapped = None


def _get_pmapped():
    global _pmapped
    if _pmapped is None:
        # x/context sharded on axis 0 (device axis); mask + weights replicated.
        _pmapped = jax.pmap(
            _forward,
            in_axes=(0, 0, None) + (None,) * len(_WNAMES),
            devices=jax.devices()[:NCORES],
        )
    return _pmapped


def kernel(**inputs) -> np.ndarray:
    x = np.asarray(inputs["x"], dtype=np.float32)
    context = np.asarray(inputs["context"], dtype=np.float32)
    mask = np.asarray(inputs["mask"])
    ws = [np.asarray(inputs[n], dtype=np.float32) for n in _WNAMES]

    bl = B // NCORES
    xs = x.reshape(NCORES, bl, N, C)
    cs = context.reshape(NCORES, bl, N, 3)

    fn = _get_pmapped()
    out = fn(xs, cs, mask, *ws)                    # [8, bl, 29, 12]
    out = np.asarray(out, dtype=np.float32).reshape(B, NMAX, D)
    return out


# revision 6
# speedup vs baseline: 6.7734x; 1.2762x over previous
"""Data-parallel EGNN message-passing kernel for 8 Trainium2 NeuronCores.

Strategy (per sharding hint): pure data parallelism. The batch B=8192 is
split across the 8 cores (1024 molecules each); the small MLP/EGNN weights
are replicated. All graph ops (kNN top-k, gather, edge MLP, aggregation,
node MLP, pooling, head) are per-molecule, so shards are independent and no
collectives are needed. Executed on the NeuronCores via jax pmap.
"""

import numpy as np
import jax
import jax.numpy as jnp
from functools import partial

B, N, C, K = 8192, 29, 6, 6
D = 2 * C            # 12
M_DIM = 32
EIN = 2 * D + 1      # 25
H = 32
IDX0, IDX1 = 0, 2
NMAX = 29
NCORES = 8


def _forward(x, context, mask, e_w1, e_b1, e_w2, e_b2, g_w, g_b,
             n_w1, n_b1, n_w2, n_b2, h_w1, h_b1, h_w2, h_b2):
    # x: [b, N, C], context: [b, N, 3], mask: [1, N] bool
    feats = jnp.concatenate([x, x], axis=-1)                      # [b,N,D]
    mask_b = jnp.broadcast_to(mask, (x.shape[0], mask.shape[1]))  # [b,N]

    coors = context
    rel = coors[:, :, None, :] - coors[:, None, :, :]             # [b,N,N,3]
    dist = jnp.sum(rel * rel, axis=-1)                            # [b,N,N]
    pair_mask = mask_b[:, :, None] & mask_b[:, None, :]
    ranking = jnp.where(pair_mask, dist, 1e5)
    # Sort-free exact top-K: iteratively extract the max of -ranking with
    # first-occurrence tie-break (matches lax.top_k semantics).
    cur = -ranking
    oh_slots = []
    for _ in range(K):
        mx = jnp.max(cur, axis=-1, keepdims=True)
        eq = (cur == mx)
        first = eq & (jnp.cumsum(eq.astype(jnp.int32), axis=-1) == 1)
        oh_slots.append(first.astype(x.dtype))
        cur = jnp.where(first, -jnp.inf, cur)
    onehot = jnp.stack(oh_slots, axis=2)                          # [b,N,K,N]
    rel_dist = jnp.einsum("bikn,bin->bik", onehot, dist)[..., None]
    # Fold edge_in concat into split matmuls: edge_in @ e_w1 =
    #   feats_i @ A + feats_j @ B + rel_dist * c   (A,B,c = rows of e_w1)
    A, Bw, cw = e_w1[:D], e_w1[D:2 * D], e_w1[2 * D:]
    U = feats @ A + e_b1                                          # [b,N,2EIN]
    V = jnp.einsum("bikn,bnh->bikh", onehot, feats @ Bw)          # [b,N,K,2EIN]
    h1 = jax.nn.silu(U[:, :, None, :] + V + rel_dist * cw[0])
    m_ij = jax.nn.silu(h1 @ e_w2 + e_b2)
    m_ij = m_ij * jax.nn.sigmoid(m_ij @ g_w + g_b)
    mask_j = jnp.einsum("bikn,bn->bik", onehot, mask_b.astype(feats.dtype)) > 0.5
    edge_mask = mask_b[:, :, None] & mask_j
    m_ij = jnp.where(edge_mask[..., None], m_ij, 0.0)
    m_i = jnp.sum(m_ij, axis=-2)                                  # [b,N,M]
    # node_in concat folded likewise.
    node_pre = feats @ n_w1[:D] + m_i @ n_w1[D:] + n_b1
    feats = jax.nn.silu(node_pre) @ n_w2 + n_b2 + feats

    mf = mask_b.astype(feats.dtype)
    pooled = jnp.sum(feats * mf[..., None], axis=1) / jnp.sum(mf, axis=1, keepdims=True)
    out = jax.nn.relu(pooled @ h_w1 + h_b1) @ h_w2 + h_b2          # [b, 2*D]
    # Only rows IDX0:IDX1 of the padded [b,29,D] output are nonzero; return
    # just them and pad on host (14x less device->host traffic).
    return out.reshape(x.shape[0], IDX1 - IDX0, D)


_WNAMES = ("e_w1", "e_b1", "e_w2", "e_b2", "g_w", "g_b",
           "n_w1", "n_b1", "n_w2", "n_b2", "h_w1", "h_b1", "h_w2", "h_b2")

_pmapped = None


def _get_pmapped():
    global _pmapped
    if _pmapped is None:
        # x/context sharded on axis 0 (device axis); mask + weights replicated.
        _pmapped = jax.pmap(
            _forward,
            in_axes=(0, 0, None) + (None,) * len(_WNAMES),
            devices=jax.devices()[:NCORES],
        )
    return _pmapped


def kernel(**inputs) -> np.ndarray:
    x = np.asarray(inputs["x"], dtype=np.float32)
    context = np.asarray(inputs["context"], dtype=np.float32)
    mask = np.asarray(inputs["mask"])
    ws = [np.asarray(inputs[n], dtype=np.float32) for n in _WNAMES]

    bl = B // NCORES
    xs = x.reshape(NCORES, bl, N, C)
    cs = context.reshape(NCORES, bl, N, 3)

    fn = _get_pmapped()
    out = fn(xs, cs, mask, *ws)                    # [8, bl, 2, 12]
    core = np.asarray(out, dtype=np.float32).reshape(B, IDX1 - IDX0, D)
    full = np.zeros((B, NMAX, D), dtype=np.float32)
    full[:, IDX0:IDX1, :] = core
    return full
